# revision 25
# baseline (speedup 1.0000x reference)
"""Trainium2 Bass kernel for nn_NoPropDTEncoder (8-core data-parallel over batch).

v3 — fp8-resident redesign vs v2:
  - v2 re-streamed x from DRAM scratch twice per block (50MB/block); all 16
    DMA queues were ~85% busy -> the kernel was HBM-bound (~300MB traffic).
  - v3 keeps x^T resident in SBUF as fp8e4 ([128, 3, 2, 32, 512], 96KB/part)
    for the scores pass, plus the last 2 groups of the s-major layout
    (24KB/part); only 6 groups of s-major fp8 x (9.4MB) stream per block.
    Total DMA drops to ~50MB f32 in + 9.4MB scratch write + 4x9.4MB reads.
  - attention matmuls (scores u.x and weighted-sum att.x) run in fp8 with
    MatmulPerfMode.DoubleRow (2 k-tiles per pass, 0.5 cycles/row): both
    operands fp8e4, PSUM f32.
  - value/dense path stays bf16 (wv, wo, classifier, pool) for precision;
    wq/wk/cden are fp8 (score path / tiny-output path).
  - pool scores computed on DVE tensor_tensor_reduce against a replicated
    pool_w (bf16), z0 weighted-sum in bf16 on PE with pre-normalized
    softmax weights.

Algorithm notes (same math as v2):
  - Single-token query attention: scores(b,h,s) = u_{b,h} . x_{b,s} with
    u = wk_h^T q_h; output needs only c = att-weighted sum of x rows.
  - FFT denoise (16 modes, per-mode complex scale) folded on host with
    proj_w into one 768x768 matrix cden.
  - eval-mode BN / biases folded on host.
"""
import sys
import os
import math

for _p in ("/opt/trn_rl_repo",):
    if _p not in sys.path and os.path.isdir(_p):
        sys.path.insert(0, _p)

import numpy as np
import concourse.bass as bass
import concourse.mybir as mybir
from concourse import tile
from concourse.bass_utils import run_bass_kernel_spmd

F32 = mybir.dt.float32
BF16 = mybir.dt.bfloat16
FP8 = mybir.dt.float8e4
WDT = BF16
DR = mybir.MatmulPerfMode.DoubleRow

B, S, D, H, DH, T, NCLS = 256, 512, 768, 8, 96, 4, 14
NCORES = 8
BL = B // NCORES          # 32 batch rows per core
DBLK = D // 128           # 6
SCH = S // 128            # 4
GB = 4                    # rows per group (PSUM col-packing at 32*b)
NG = BL // GB             # 8 groups
NRES = 0                  # groups kept resident in SBUF (the last ones)
NSTR = NG - NRES          # streamed groups
GBH = 2                   # rows per pass-0 half-group (SBUF pressure)
NGH = BL // GBH           # 16 half-groups
GP = 32 * (GB - 1) + H    # 104 live partitions in packed tiles
EPS = 1e-5
RSQD = 1.0 / math.sqrt(DH)

AF = mybir.ActivationFunctionType
ALU = mybir.AluOpType

VKEYS = ("bo_sa", "projb_sd", "ln1_g", "ln1_b", "ln2_g", "ln2_b",
         "bn_s", "bn_b")


def split_sync_waits(nc, cap=1):
    """walrus in this container rejects >1 sync wait per CTRL instruction;
    move extra waits onto NoOp carriers inserted just before (same engine,
    program order => semantically identical)."""
    for f in nc.m.functions:
        for bb in f.blocks:
            il = bb.instructions
            i = 0
            while i < len(il):
                inst = il[i]
                si = inst.sync_info
                nw = len(si.on_wait) if si is not None else 0
                if nw > cap:
                    waits = list(si.on_wait)
                    ups = list(si.on_update)
                    extra, keep = waits[:-cap], waits[-cap:]
                    for j in range(0, len(extra), cap):
                        nop = mybir.InstNoOp(
                            name=f"{inst.name}-sw{j}", engine=inst.engine,
                            sync_info=mybir.SyncInfo(on_wait=extra[j:j + cap],
                                                     on_update=[]),
                            ins=[], outs=[])
                        il.insert(i, nop)
                        i += 1
                    inst.sync_info = mybir.SyncInfo(on_wait=keep, on_update=ups)
                i += 1


# ---------------------------------------------------------------------------
# host-side weight preprocessing
# ---------------------------------------------------------------------------

def _prep_weights(inp):
    f = np.float32
    w = {}
    wq = np.asarray(inp["w_q"], f)
    wk = np.asarray(inp["w_k"], f)
    wv = np.asarray(inp["w_v"], f)
    wo = np.asarray(inp["w_o"], f)
    bqkv = np.asarray(inp["b_qkv"], f)
    proj_w = np.asarray(inp["proj_w"], f)
    proj_b = np.asarray(inp["proj_b"], f)
    sa = np.asarray(inp["scale_attn"], f)
    sd = np.asarray(inp["scale_denoise"], f)

    w["wqT"] = np.ascontiguousarray(
        np.transpose(wq, (0, 2, 1)) * RSQD).astype(f)           # [T, j, i]
    bq = bqkv[:, :D] * RSQD                                     # [T, 768]
    w["bq"] = np.ascontiguousarray(
        bq.reshape(T, H, DH).transpose(0, 2, 1)).astype(f)      # [T, 96, 8]
    w["wk"] = np.ascontiguousarray(wk).astype(f)                # [T, (h i), j]
    w["wvT"] = np.ascontiguousarray(np.transpose(wv, (0, 2, 1)))
    w["woT"] = np.ascontiguousarray(np.transpose(wo, (0, 2, 1)))  # [T, m, k]
    bv = bqkv[:, 2 * D:]
    bo_eff = np.asarray(inp["b_o"], f) + np.einsum("tkm,tm->tk", wo, bv)
    bo_sa = (sa[:, None] * bo_eff).astype(f)                    # [T, 768]

    # denoise: z_spatial = z1 @ M_t ; fold with proj -> C = M @ proj_w.T
    sr = np.asarray(inp["scale_real"], np.float64)
    si = np.asarray(inp["scale_imag"], np.float64)
    filt = np.mean(sr + 1j * si, axis=2)                        # [T, 16]
    jj = np.arange(D)
    mm = np.arange(16)
    W1 = np.exp(-2j * np.pi * np.outer(jj, mm) / D)             # [768, 16]
    W2 = np.exp(+2j * np.pi * np.outer(mm, jj) / D)             # [16, 768]
    cden = np.empty((T, D, D), np.float32)
    for t in range(T):
        Mt = np.real(W1 @ (filt[t][:, None] * W2)) / D          # [j, j']
        cden[t] = (Mt @ proj_w[t].T.astype(np.float64)).astype(np.float32)
    w["cden"] = np.ascontiguousarray(cden * sd[:, None, None])  # [T, j, k]
    projb_sd = (sd[:, None] * proj_b).astype(f)

    s = 1.0 / math.sqrt(1.0 + EPS)
    vec = {
        "bo_sa": bo_sa, "projb_sd": projb_sd,
        "ln1_g": np.asarray(inp["ln1_g"], f), "ln1_b": np.asarray(inp["ln1_b"], f),
        "ln2_g": np.asarray(inp["ln2_g"], f), "ln2_b": np.asarray(inp["ln2_b"], f),
        "bn_s": (np.asarray(inp["bn_g"], f) * s).astype(f),
        "bn_b": np.asarray(inp["bn_b"], f),
    }
    w["vecs8"] = np.ascontiguousarray(
        np.stack([vec[k] for k in VKEYS], axis=1))              # [T, 8, 768]

    f1s = (np.asarray(inp["f1_bn_g"], f) * s).astype(f)
    w["f1w"] = np.asarray(inp["f1_w"], f)
    w["f1s"] = f1s
    w["f1b"] = (np.asarray(inp["f1_b"], f) * f1s + np.asarray(inp["f1_bn_b"], f))
    f2s = (np.asarray(inp["f2_bn_g"], f) * s).astype(f)
    w["f2w"] = np.asarray(inp["f2_w"], f)
    w["f2s"] = f2s
    w["f2b"] = (np.asarray(inp["f2_b"], f) * f2s + np.asarray(inp["f2_bn_b"], f))
    w["clsw"] = np.asarray(inp["cls_w"], f)
    w["clsb"] = np.asarray(inp["cls_b"], f)

    w["poolwrep"] = np.ascontiguousarray(
        np.tile(np.asarray(inp["pool_w"], f).reshape(1, D), (128, 1)))
    w["identb"] = np.eye(128, dtype=np.float32)

    g = 1.0 / (1.0 + np.exp(-np.asarray(inp["gate"], np.float64)))
    scal = {"g": [float(v) for v in g],
            "sa": [float(v) for v in sa],
            "sd": [float(v) for v in sd]}
    return w, scal


def _np_dt(dt):
    return mybir.dt.np(dt)


# ---------------------------------------------------------------------------
# program builder
# ---------------------------------------------------------------------------

def build_program(scal, for_sim=False):
    nc = bass.Bass()
    P = {}
    P["x"] = nc.declare_dram_parameter("x", [BL, S, D], F32, isOutput=False)
    P["identb"] = nc.declare_dram_parameter("identb", [128, 128], BF16, isOutput=False)
    P["poolwrep"] = nc.declare_dram_parameter("poolwrep", [128, D], BF16, isOutput=False)
    P["wqT"] = nc.declare_dram_parameter("wqT", [T, D, D], FP8, isOutput=False)
    P["bq"] = nc.declare_dram_parameter("bq", [T, DH, H], F32, isOutput=False)
    P["wk"] = nc.declare_dram_parameter("wk", [T, D, D], FP8, isOutput=False)
    P["wvT"] = nc.declare_dram_parameter("wvT", [T, D, D], WDT, isOutput=False)
    P["woT"] = nc.declare_dram_parameter("woT", [T, D, D], WDT, isOutput=False)
    P["cden"] = nc.declare_dram_parameter("cden", [T, D, D], FP8, isOutput=False)
    P["vecs8"] = nc.declare_dram_parameter("vecs8", [T, 8, D], F32, isOutput=False)
    P["f1w"] = nc.declare_dram_parameter("f1w", [D, 256], WDT, isOutput=False)
    P["f1s"] = nc.declare_dram_parameter("f1s", [256], F32, isOutput=False)
    P["f1b"] = nc.declare_dram_parameter("f1b", [256], F32, isOutput=False)
    P["f2w"] = nc.declare_dram_parameter("f2w", [256, 128], WDT, isOutput=False)
    P["f2s"] = nc.declare_dram_parameter("f2s", [128], F32, isOutput=False)
    P["f2b"] = nc.declare_dram_parameter("f2b", [128], F32, isOutput=False)
    P["clsw"] = nc.declare_dram_parameter("clsw", [128, NCLS], WDT, isOutput=False)
    P["clsb"] = nc.declare_dram_parameter("clsb", [NCLS], F32, isOutput=False)
    P["out"] = nc.declare_dram_parameter("out", [NCLS, BL], F32, isOutput=True)

    with tile.TileContext(nc) as tc:
        _body(nc, tc, P, scal, for_sim)
    if not for_sim:
        split_sync_waits(nc)
    return nc


def _body(nc, tc, P, scal, for_sim=False):
    import contextlib

    def sim_init(ap):
        # CoreSim tracks uninitialized memory; partially-written PSUM tiles
        # whose garbage partitions are read-but-never-consumed are fine on
        # HW but assert in sim. Zero them in the sim build only.
        if for_sim:
            nc.vector.memset(ap, 0.0)
    ctx = contextlib.ExitStack()
    pool_c = ctx.enter_context(tc.tile_pool(name="consts", bufs=1))
    pool_w = ctx.enter_context(tc.tile_pool(name="weights", bufs=1))
    pool_x = ctx.enter_context(tc.tile_pool(name="x", bufs=2))
    pool_s = ctx.enter_context(tc.tile_pool(name="state", bufs=1))
    pool_t = ctx.enter_context(tc.tile_pool(name="tmp", bufs=2))
    psum = ctx.enter_context(tc.tile_pool(name="ps", bufs=2,
                                          space=bass.MemorySpace.PSUM))
    pool_d = ctx.enter_context(tc.tile_pool(name="dram", bufs=1,
                                            space=bass.MemorySpace.DRAM))

    # s-major fp8 scratch for the streamed groups (layout matches the SBUF
    # tile verbatim: [128 s_lo, b, sc, d])
    xn_scr = pool_d.tile([NSTR, 128, GB, SCH, D], FP8, tag="xn_scr")

    def dma(dst, src):
        # SWDGE: this container's walrus rejects HWDGE trigger ISA structs
        nc.gpsimd.dma_start(out=dst, in_=src)

    def dma_cast(dst, src):
        # SWDGE (gpsimd) converts dtype in flight
        nc.gpsimd.dma_start(out=dst, in_=src)

    # ---- constants -------------------------------------------------------
    identb = pool_c.tile([128, 128], BF16, tag="identb")
    dma_cast(identb[:], P["identb"][:])
    identf8 = pool_c.tile([128, 128], FP8, tag="identf8")
    nc.vector.tensor_copy(identf8[:], identb[:])
    poolwrep = pool_c.tile([128, D], BF16, tag="poolwrep")
    dma(poolwrep[:], P["poolwrep"][:])
    onescol = pool_c.tile([128, 1], F32, tag="ones")
    nc.vector.memset(onescol[:], 1.0 / D)
    onescolb = pool_c.tile([128, 1], BF16, tag="onesb")
    nc.vector.memset(onescolb[:], 1.0)
    onesrow = pool_c.tile([1, 128], F32, tag="onesrow")
    nc.vector.memset(onesrow[:], 1.0)
    zeroc = pool_c.tile([128, 1], F32, tag="zeroc")
    nc.vector.memset(zeroc[:], 0.0)
    nc.const_aps.aps[(F32, 0.0)] = zeroc[:]
    epsc = pool_c.tile([128, 1], F32, tag="epsc")
    nc.vector.memset(epsc[:], EPS)
    nc.const_aps.aps[(F32, EPS)] = epsc[:]

    # classifier weights (loaded once)
    f1w = pool_c.tile([128, DBLK, 256], WDT, tag="f1w")
    dma(f1w[:], P["f1w"].rearrange("(c p) n -> p c n", p=128))
    f2w = pool_c.tile([128, 2, 128], WDT, tag="f2w")
    dma(f2w[:], P["f2w"].rearrange("(c p) n -> p c n", p=128))
    clsw = pool_c.tile([128, NCLS], WDT, tag="clsw")
    dma(clsw[:], P["clsw"][:])
    f1s = pool_c.tile([128, 2], F32, tag="f1s")
    dma(f1s[:], P["f1s"].rearrange("(c p) -> p c", p=128))
    f1b = pool_c.tile([128, 2], F32, tag="f1b")
    dma(f1b[:], P["f1b"].rearrange("(c p) -> p c", p=128))
    f2s = pool_c.tile([128, 1], F32, tag="f2s")
    dma(f2s[:], P["f2s"].rearrange("(c p) -> p c", p=128))
    f2b = pool_c.tile([128, 1], F32, tag="f2b")
    dma(f2b[:], P["f2b"].rearrange("(c p) -> p c", p=128))
    clsb = pool_c.tile([NCLS, 1], F32, tag="clsb")
    dma(clsb[:], P["clsb"].rearrange("(p c) -> p c", c=1))

    # persistent state
    # x^T resident: XT[p, pr, i, b, s] = x[b, s, (2*pr+i)*128 + p]
    XT = pool_s.tile([128, 3, 2, BL, S], FP8, tag="XT")
    # s-major resident groups (the last NRES): [p, j, b_in_g, sc, d]
    xnres = (pool_s.tile([128, NRES, GB, SCH, D], FP8, tag="xnres")
             if NRES else None)
    zT = pool_s.tile([128, BL, DBLK], F32, tag="zT")
    zTb = pool_s.tile([128, BL, DBLK], WDT, tag="zTb")
    logitsT = pool_s.tile([NCLS, BL], F32, tag="logits")

    # =====================================================================
    # pass 0: stream x f32->bf16 in half-groups (2 batch rows); build fp8 XT
    # (PE transposes) + fp8 s-major scratch; attention pool -> z0 (bf16
    # path for precision).
    # =====================================================================
    p0_xn = {}
    GPH = 32 * (GBH - 1) + 1  # live packed partitions for z0 rows

    def p0_load(ci):
        rows = slice(ci * GBH, (ci + 1) * GBH)
        xn = pool_x.tile([128, GBH, SCH, D], BF16, tag="xng", bufs=2)
        nc.gpsimd.dma_start(
            out=xn[:], in_=P["x"][rows].rearrange("b (sc p) d -> p b sc d", p=128))
        p0_xn[ci] = xn

    def p0_store(ci):
        xn = p0_xn[ci]
        gi, h = ci * GBH // GB, (ci * GBH) % GB
        if gi < NSTR:
            dma_cast(xn_scr[gi][:, h:h + GBH], xn[:])  # bf16 -> fp8 out
        else:
            nc.vector.tensor_copy(xnres[:, gi - NSTR, h:h + GBH], xn[:])

    def p0_pool(ci):
        rows = slice(ci * GBH, (ci + 1) * GBH)
        xn = p0_xn[ci]
        # scores: product on DVE, free-dim reduction via Scalar accumulate
        # (TensorTensorReduce is not encodable by this container's walrus)
        pscore = pool_t.tile([128, GBH, SCH], F32, tag="pscore", bufs=1)
        for b in range(GBH):
            for sc in range(SCH):
                scr = pool_t.tile([128, D], BF16, tag="ttr_scr", bufs=2)
                nc.gpsimd.tensor_mul(scr[:], xn[:, b, sc, :], poolwrep[:])
                nc.scalar.activation(scr[:], scr[:], AF.Identity,
                                     accum_out=pscore[:, b, sc:sc + 1])
        pes = pool_t.tile([128, GBH, SCH], BF16, tag="pes", bufs=1)
        nc.scalar.activation(pes[:], pscore[:], AF.Exp)
        # denominators: sum over s_lo partitions (PE) then over sc (DVE)
        denp = psum.tile([1, GBH, SCH], F32, tag="lnm", bufs=1, name="denp")
        nc.tensor.matmul(denp.rearrange("p b sc -> p (b sc)"), onescolb[:],
                         pes.rearrange("p b sc -> p (b sc)"),
                         start=True, stop=True)
        dcp = pool_t.tile([1, GBH, SCH], F32, tag="dcp", bufs=1)
        nc.vector.tensor_copy(dcp[:], denp[:])
        dsum = pool_t.tile([1, GBH], F32, tag="dsum", bufs=1)
        nc.vector.tensor_add(dsum[:], dcp[:, :, 0], dcp[:, :, 1])
        nc.vector.tensor_add(dsum[:], dsum[:], dcp[:, :, 2])
        nc.vector.tensor_add(dsum[:], dsum[:], dcp[:, :, 3])
        drec = pool_t.tile([1, GBH], F32, tag="drec", bufs=1)
        nc.vector.reciprocal(drec[:], dsum[:])
        bcp = psum.tile([128, GBH], F32, tag="lnm", bufs=1, name="bcp")
        nc.tensor.matmul(bcp[:], onesrow[:], drec[:], start=True, stop=True)
        bc = pool_t.tile([128, GBH], F32, tag="pbc", bufs=1)
        nc.vector.tensor_copy(bc[:], bcp[:])
        pesn = pool_t.tile([128, GBH, SCH], BF16, tag="pesn", bufs=1)
        for b in range(GBH):
            nc.vector.tensor_scalar(pesn[:, b, :], pes[:, b, :],
                                    bc[:, b:b + 1], None, op0=ALU.mult)
        # z0 rows at partitions 32*b (m=1 weighted sums, pre-normalized)
        z1p = psum.tile([GP, 512], F32, tag="ws1", name="z1p")
        z2p = psum.tile([GP, 256], F32, tag="ws2", bufs=1, name="z2p")
        sim_init(z1p[:])
        sim_init(z2p[:])
        for b in range(GBH):
            for sc in range(SCH):
                nc.tensor.matmul(z1p[32 * b:32 * b + 1, :],
                                 pesn[:, b, sc:sc + 1], xn[:, b, sc, 0:512],
                                 start=(sc == 0), stop=(sc == SCH - 1),
                                 tile_position=(0, 32 * b))
                nc.tensor.matmul(z2p[32 * b:32 * b + 1, :],
                                 pesn[:, b, sc:sc + 1], xn[:, b, sc, 512:D],
                                 start=(sc == 0), stop=(sc == SCH - 1),
                                 tile_position=(0, 32 * b))
        zn = pool_t.tile([GPH, D], BF16, tag="zn", bufs=1)
        nc.vector.tensor_copy(zn[:, 0:512], z1p[0:GPH, :])
        nc.vector.tensor_copy(zn[:, 512:D], z2p[0:GPH, :])
        ztp = psum.tile([128, DBLK, GPH + 3], BF16, tag="sc", name="ztp")
        for dc in range(DBLK):
            nc.tensor.transpose(ztp[:, dc, 0:GPH], zn[:, dc * 128:(dc + 1) * 128],
                                identb[0:GPH, 0:GPH])
        src = ztp[:, :, 0:GPH:32].rearrange("p dc b -> p b dc")  # [128, 2, 6]
        nc.vector.tensor_copy(zT[:, rows, :], src)
        nc.vector.tensor_copy(zTb[:, rows, :], src)

    # block weight loads
    wqs, wks, bqvs, vecs = {}, {}, {}, {}

    def emit_weights(t):
        wq = pool_w.tile([128, DBLK, D], FP8, tag="wq", name="wq")
        dma(wq[:], P["wqT"][t].rearrange("(c p) n -> p c n", p=128))
        wk = pool_w.tile([DH, H, D], FP8, tag="wk", name="wk")
        dma(wk[:], P["wk"][t].rearrange("(h p) n -> p h n", p=DH))
        bqv = pool_w.tile([DH, H], F32, tag="bqv")
        dma(bqv[:], P["bq"][t])
        vec = pool_w.tile([128, 8, DBLK], F32, tag="vecs")
        dma(vec[:], P["vecs8"][t].rearrange("v (c p) -> p v c", p=128))
        wqs[t], wks[t], bqvs[t], vecs[t] = wq, wk, bqv, vec

    p0_load(0)
    emit_weights(0)
    for ci in range(NGH):
        if ci + 1 < NGH:
            p0_load(ci + 1)
        p0_store(ci)
        p0_pool(ci)

    # =====================================================================
    # transformer blocks
    # =====================================================================
    # group order: resident groups first so stream DMA has lead time
    ORDER = list(range(NSTR, NG)) + list(range(NSTR))
    xn8s = {}

    def emit_load(t, gi):
        # gi is a streamed group id (< NSTR)
        xn8 = pool_x.tile([128, GB, SCH, D], FP8, tag="xn8", bufs=2)
        dma(xn8[:], xn_scr[gi])
        xn8s[(t, gi)] = xn8

    def xn_of(t, gi, pop=True):
        if gi >= NSTR:
            return xnres[:, gi - NSTR]
        return (xn8s.pop((t, gi)) if pop else xn8s[(t, gi)])[:]

    pending_cls = [None]

    for t in range(T):
        g = scal["g"][t]
        sa = scal["sa"][t]

        wq, wk, bqv, vec = wqs[t], wks[t], bqvs[t], vecs[t]

        def vslice(k, dc, vec=vec):
            return vec[:, VKEYS.index(k), dc:dc + 1]

        # prefetch the first streamed groups
        emit_load(t, 0)
        emit_load(t, 1)

        # --- q^T = wqT^T @ zT + bq  -> [96, 8, 32] ---
        qp = psum.tile([DH, H, BL], F32, tag="mm")
        for h in range(H):
            for jc in range(DBLK):
                nc.tensor.matmul(qp[:, h, :],
                                 wq[:, jc, h * DH:(h + 1) * DH],
                                 zTb[:, :, jc],
                                 start=(jc == 0), stop=(jc == DBLK - 1))
        qT = pool_t.tile([DH, H, BL], WDT, tag="qT", bufs=1)
        for h in range(H):
            nc.vector.tensor_scalar_add(qT[:, h, :], qp[:, h, :], bqv[:, h:h + 1])

        # --- u = wk_h^T q_h -> fp8 [128, 3, 2, 32, 8] (b-major for 8B-aligned
        # DoubleRow weight slices) ---
        uT = pool_t.tile([128, 3, 2, BL, H], FP8, tag="uT", bufs=1)
        for pr in range(3):
            up = psum.tile([128, 2, H, BL], F32, tag="mm")
            for i in range(2):
                dc = 2 * pr + i
                for h in range(H):
                    nc.tensor.matmul(up[:, i, h, :],
                                     wk[:, h, dc * 128:(dc + 1) * 128],
                                     qT[:, h, :], start=True, stop=True)
            nc.vector.tensor_copy(
                uT[:, pr].rearrange("p i b h -> p i h b"), up[:])

        # previous block's classifier rides the group phase (PE slack)
        if pending_cls[0] is not None:
            pending_cls[0]()
            pending_cls[0] = None

        # prefetch tail weights + next block's head weights
        wv = pool_w.tile([128, DBLK, D], WDT, tag="wv", name="wv")
        dma(wv[:], P["wvT"][t].rearrange("(c p) n -> p c n", p=128))
        wo = pool_w.tile([DH, H, D], WDT, tag="wo", name="wo")
        dma(wo[:], P["woT"][t].rearrange("(h p) n -> p h n", p=DH))
        cdn = pool_w.tile([128, DBLK, D], FP8, tag="cd", name="cdn")
        dma(cdn[:], P["cden"][t].rearrange("(c p) n -> p c n", p=128))
        if t + 1 < T:
            emit_weights(t + 1)

        # --- streaming pass over x: software-pipelined group loop ---
        cT = pool_t.tile([128, DBLK, H, BL], WDT, tag="cT", bufs=1)
        ess, dens, esTs, chs = {}, {}, {}, {}

        def emit_xt(gi):
            # block 0 only: build resident fp8 XT from the fp8 stream tiles
            # (fp8 PE transpose writes PSUM with element step 2)
            xn8 = xn_of(0, gi, pop=False)
            for bb in range(GB):
                b = gi * GB + bb
                for sc in range(SCH):
                    tp8 = psum.tile([128, DBLK, 256], FP8, tag="mm",
                                    name="tp8")
                    for dc in range(DBLK):
                        nc.tensor.transpose(
                            tp8[:, dc, 0:256:2],
                            xn8[:, bb, sc, dc * 128:(dc + 1) * 128],
                            identf8[:])
                    nc.vector.tensor_copy(
                        XT[:, :, :, b, sc * 128:(sc + 1) * 128],
                        tp8[:, :, 0:256:2].rearrange(
                            "p (pr i) s -> p pr i s", pr=3))

        def emit_scores(gi):
            ps = psum.tile([GP, S], F32, tag="sc")
            sim_init(ps[:])
            for bb in range(GB):
                b = gi * GB + bb
                for dc in range(DBLK):
                    nc.tensor.matmul(ps[32 * bb:32 * bb + H, :],
                                     uT[:, dc >> 1, dc & 1, b, :],
                                     XT[:, dc >> 1, dc & 1, b, :],
                                     start=(dc == 0), stop=(dc == DBLK - 1),
                                     tile_position=(0, 32 * bb))
            es = pool_t.tile([GP, S], BF16, tag="es")
            den = pool_t.tile([GP, 2], F32, tag="den")
            nc.scalar.activation(es[:], ps[:], AF.Exp, accum_out=den[:, 0:1])
            nc.vector.reciprocal(den[:, 1:2], den[:, 0:1])
            ess[gi], dens[gi] = es, den

        def emit_attT(gi):
            es = ess.pop(gi)
            esTp = psum.tile([128, SCH, GP], BF16, tag="lnm", bufs=1)
            for sc in range(SCH):
                nc.tensor.transpose(esTp[:, sc, 0:GP],
                                    es[:, sc * 128:(sc + 1) * 128],
                                    identb[0:GP, 0:GP])
            esT = pool_t.tile([128, 2, 2, GP], FP8, tag="esT")
            nc.vector.tensor_copy(
                esT[:], esTp.rearrange("p (pr i) g -> p pr i g", pr=2))
            esTs[gi] = esT

        def emit_wsum(t, gi):
            xn8 = xn_of(t, gi)
            xv = xn8.rearrange("p b (pr i) d -> p b pr i d", pr=2)
            esT, den = esTs.pop(gi), dens.pop(gi)
            c1 = psum.tile([GP, 512], F32, tag="ws1")
            c2 = psum.tile([GP, 256], F32, tag="ws2", bufs=1)
            sim_init(c1[:])
            sim_init(c2[:])
            for bb in range(GB):
                for sc in range(SCH):
                    nc.tensor.matmul(c1[32 * bb:32 * bb + H, :],
                                     esT[:, sc >> 1, sc & 1, 32 * bb:32 * bb + H],
                                     xv[:, bb, sc >> 1, sc & 1, 0:512],
                                     start=(sc == 0), stop=(sc == SCH - 1),
                                     tile_position=(0, 32 * bb))
                    nc.tensor.matmul(c2[32 * bb:32 * bb + H, :],
                                     esT[:, sc >> 1, sc & 1, 32 * bb:32 * bb + H],
                                     xv[:, bb, sc >> 1, sc & 1, 512:D],
                                     start=(sc == 0), stop=(sc == SCH - 1),
                                     tile_position=(0, 32 * bb))
            ch = pool_t.tile([GP, D], BF16, tag="ch")
            nc.vector.tensor_scalar_mul(ch[:, 0:512], c1[:], den[:, 1:2])
            nc.vector.tensor_scalar_mul(ch[:, 512:D], c2[:], den[:, 1:2])
            chs[gi] = ch

        def emit_ctpT(gi):
            ch = chs.pop(gi)
            ctpp = psum.tile([128, DBLK, 128], BF16, tag="mm")
            for dc in range(DBLK):
                nc.tensor.transpose(ctpp[:, dc, 0:GP],
                                    ch[:, dc * 128:(dc + 1) * 128],
                                    identb[0:GP, 0:GP])
            # scatter (dc, 32*bb+h) -> cT[:, dc, h, gi*GB+bb]
            src = ctpp.rearrange("p dc (b r) -> p dc b r", r=32)[:, :, :, 0:H]
            nc.vector.tensor_copy(
                cT.rearrange("p dc h bl -> p dc bl h")[:, :, gi * GB:(gi + 1) * GB, :],
                src)

        for j, gi in enumerate(ORDER):
            if t == 0:
                emit_xt(gi)
            emit_scores(gi)
            if j >= 1:
                emit_wsum(t, ORDER[j - 1])
            emit_attT(gi)
            if j >= 1:
                emit_ctpT(ORDER[j - 1])
            if j + 2 < NG and j + 2 >= NRES:
                emit_load(t, ORDER[j + 2])
        emit_wsum(t, ORDER[NG - 1])
        emit_ctpT(ORDER[NG - 1])

        # --- o_h = wvT_h^T @ c_h -> [96, 8, 32] bf16 ---
        op = psum.tile([DH, H, BL], F32, tag="mm")
        for h in range(H):
            for jc in range(DBLK):
                nc.tensor.matmul(op[:, h, :],
                                 wv[:, jc, h * DH:(h + 1) * DH],
                                 cT[:, jc, h, :],
                                 start=(jc == 0), stop=(jc == DBLK - 1))
        oT = pool_t.tile([DH, H, BL], WDT, tag="oT", bufs=1)
        nc.vector.tensor_copy(oT[:], op[:])

        # --- z_attn^T = woT^T @ o ; y = z + sa*z_attn + sa*bo_eff ---
        zap = psum.tile([128, DBLK, BL], F32, tag="mm")
        for mk in range(DBLK):
            for h in range(H):
                nc.tensor.matmul(zap[:, mk, :],
                                 wo[:, h, mk * 128:(mk + 1) * 128],
                                 oT[:, h, :], start=(h == 0), stop=(h == H - 1))
        yT = pool_s.tile([128, BL, DBLK], F32, tag="yT")
        for mk in range(DBLK):
            nc.vector.tensor_scalar(yT[:, :, mk], zap[:, mk, :],
                                    sa, vslice("bo_sa", mk),
                                    op0=ALU.mult, op1=ALU.add)
        nc.vector.tensor_add(yT[:], yT[:], zT[:])

        # --- LN1 -> z1T ---
        z1T = pool_s.tile([128, BL, DBLK], F32, tag="z1T")
        _layernorm(nc, tc, psum, pool_t, yT, z1T, onescol, onesrow,
                   lambda dc: vslice("ln1_g", dc), lambda dc: vslice("ln1_b", dc))

        # --- denoise: z_den^T = cden^T @ z1T ; y2 = z1 + sd*(...) ---
        z1Tb = pool_t.tile([128, BL, DBLK], WDT, tag="z1Tb", bufs=1)
        nc.vector.tensor_copy(z1Tb[:], z1T[:])
        dp = psum.tile([128, DBLK, BL], F32, tag="mm")
        for mk in range(DBLK):
            for jc in range(DBLK):
                nc.tensor.matmul(dp[:, mk, :],
                                 cdn[:, jc, mk * 128:(mk + 1) * 128],
                                 z1Tb[:, :, jc],
                                 start=(jc == 0), stop=(jc == DBLK - 1))
        y2T = pool_s.tile([128, BL, DBLK], F32, tag="yT")  # reuse yT slot
        for mk in range(DBLK):
            nc.vector.tensor_scalar_add(y2T[:, :, mk], dp[:, mk, :],
                                        vslice("projb_sd", mk))
        nc.vector.tensor_add(y2T[:], y2T[:], z1T[:])

        # --- LN2 -> z2T ---
        z2T = pool_s.tile([128, BL, DBLK], F32, tag="z1T")  # reuse z1T slot
        _layernorm(nc, tc, psum, pool_t, y2T, z2T, onescol, onesrow,
                   lambda dc: vslice("ln2_g", dc), lambda dc: vslice("ln2_b", dc))

        # --- gate mix + BN -> new z ---
        nc.vector.tensor_sub(z2T[:], z2T[:], zT[:])       # z2 - z
        nc.vector.tensor_scalar(z2T[:], z2T[:], g, None,
                                op0=ALU.mult)              # g*(z2-z)
        nc.vector.tensor_add(z2T[:], z2T[:], zT[:])       # + z
        for dc in range(DBLK):
            nc.vector.tensor_scalar(zT[:, :, dc], z2T[:, :, dc],
                                    vslice("bn_s", dc), vslice("bn_b", dc),
                                    op0=ALU.mult, op1=ALU.add)
        nc.vector.tensor_copy(zTb[:], zT[:])

        # --- classifier (deferred: emitted during block t+1's group phase) ---
        def emit_classifier(t=t):
            hp = psum.tile([128, 2, BL], F32, tag="mm")
            for mk in range(2):
                for jc in range(DBLK):
                    nc.tensor.matmul(hp[:, mk, :],
                                     f1w[:, jc, mk * 128:(mk + 1) * 128],
                                     zTb[:, :, jc],
                                     start=(jc == 0), stop=(jc == DBLK - 1))
            h1 = pool_t.tile([128, 2, BL], WDT, tag="h1", bufs=1)
            for mk in range(2):
                nc.scalar.activation(h1[:, mk, :], hp[:, mk, :], AF.Relu,
                                     bias=f1b[:, mk:mk + 1],
                                     scale=f1s[:, mk:mk + 1])
            h2p = psum.tile([128, BL], F32, tag="mm")
            for jc in range(2):
                nc.tensor.matmul(h2p[:], f2w[:, jc, :], h1[:, jc, :],
                                 start=(jc == 0), stop=(jc == 1))
            h2 = pool_t.tile([128, BL], WDT, tag="h2", bufs=1)
            nc.scalar.activation(h2[:], h2p[:], AF.Relu,
                                 bias=f2b[:, 0:1], scale=f2s[:, 0:1])
            lp = psum.tile([NCLS, BL], F32, tag="mm")
            nc.tensor.matmul(lp[:], clsw[:], h2[:], start=True, stop=True)
            if t == 0:
                nc.vector.tensor_copy(logitsT[:], lp[:])
            else:
                nc.vector.tensor_add(logitsT[:], logitsT[:], lp[:])

        pending_cls[0] = emit_classifier

    pending_cls[0]()
    pending_cls[0] = None

    # --- epilogue: /T + cls_b, store ---
    outt = pool_t.tile([NCLS, BL], F32, tag="outt")
    nc.scalar.activation(outt[:], logitsT[:], AF.Identity,
                         bias=clsb[:, 0:1], scale=1.0 / T)
    dma_cast(P["out"][:], outt[:])
    ctx.close()


def _layernorm(nc, tc, psum, pool_t, yT, outT, onescol, onesrow, gf, bf):
    """T-layout layernorm over d (partition x dblk); DVE + PE reductions."""
    mp = psum.tile([1, BL], F32, tag="lnm", bufs=1, name="mp")
    m2p = psum.tile([1, BL], F32, tag="mm", name="m2p")
    sq = pool_t.tile([128, BL, DBLK], F32, tag="ln_sq", bufs=1)
    nc.vector.tensor_mul(sq[:], yT[:], yT[:])
    for dc in range(DBLK):
        nc.tensor.matmul(mp[:], onescol[:], yT[:, :, dc],
                         start=(dc == 0), stop=(dc == DBLK - 1))
        nc.tensor.matmul(m2p[:], onescol[:], sq[:, :, dc],
                         start=(dc == 0), stop=(dc == DBLK - 1))
    st = pool_t.tile([1, 2 * BL], F32, tag="ln_st", bufs=1)  # [mu | rstd]
    nc.vector.tensor_copy(st[:, 0:BL], mp[:])
    mu2 = pool_t.tile([1, BL], F32, tag="ln_mu2", bufs=1)
    nc.vector.tensor_mul(mu2[:], st[:, 0:BL], st[:, 0:BL])
    var = pool_t.tile([1, BL], F32, tag="ln_var", bufs=1)
    nc.vector.tensor_sub(var[:], m2p[:], mu2[:])
    nc.scalar.activation(var[:], var[:], AF.Sqrt, bias=EPS)
    nc.vector.reciprocal(st[:, BL:2 * BL], var[:])
    bcp = psum.tile([128, 2 * BL], F32, tag="lnm", bufs=1)
    nc.tensor.matmul(bcp[:], onesrow[:], st[:], start=True, stop=True)
    bc = pool_t.tile([128, 2 * BL], F32, tag="ln_bc", bufs=1)
    nc.vector.tensor_copy(bc[:], bcp[:])
    mub = bc[:, 0:BL]
    rsb = bc[:, BL:2 * BL]
    for dc in range(DBLK):
        nc.vector.tensor_sub(outT[:, :, dc], yT[:, :, dc], mub)
        nc.vector.tensor_mul(outT[:, :, dc], outT[:, :, dc], rsb)
        nc.vector.tensor_scalar(outT[:, :, dc], outT[:, :, dc],
                                gf(dc), bf(dc), op0=ALU.mult, op1=ALU.add)


# ---------------------------------------------------------------------------
# entry point
# ---------------------------------------------------------------------------

_PROG_CACHE = {}


def _make_in_maps(inputs, w, scal):
    x = np.asarray(inputs["x_feat"], np.float32)
    assert x.shape == (B, S, D), x.shape

    def cast(a, dt):
        return np.ascontiguousarray(a).astype(_np_dt(dt))

    shared = {
        "identb": cast(w["identb"], BF16),
        "poolwrep": cast(w["poolwrep"], BF16),
        "wqT": cast(w["wqT"], FP8), "bq": w["bq"], "wk": cast(w["wk"], FP8),
        "wvT": cast(w["wvT"], WDT), "woT": cast(w["woT"], WDT),
        "cden": cast(w["cden"], FP8),
        "vecs8": w["vecs8"],
        "f1w": cast(w["f1w"], WDT), "f1s": w["f1s"], "f1b": w["f1b"],
        "f2w": cast(w["f2w"], WDT), "f2s": w["f2s"], "f2b": w["f2b"],
        "clsw": cast(w["clsw"], WDT), "clsb": w["clsb"],
    }
    in_maps = []
    for c in range(NCORES):
        m = dict(shared)
        m["x"] = np.ascontiguousarray(x[c * BL:(c + 1) * BL])
        in_maps.append(m)
    return in_maps


LAST_EXEC_NS = None
LAST_RESULTS = None


def kernel(**inputs):
    global LAST_EXEC_NS, LAST_RESULTS
    w, scal = _prep_weights(inputs)
    key = tuple(scal["g"]) + tuple(scal["sa"]) + tuple(scal["sd"])
    if key not in _PROG_CACHE:
        _PROG_CACHE[key] = build_program(scal)
    nc = _PROG_CACHE[key]
    in_maps = _make_in_maps(inputs, w, scal)
    res = run_bass_kernel_spmd(nc, in_maps, core_ids=list(range(NCORES)))
    LAST_RESULTS = res
    if res.exec_time_ns:
        LAST_EXEC_NS = res.exec_time_ns
    out = np.concatenate(
        [np.asarray(res.results[c]["out"]).T for c in range(NCORES)], axis=0)
    return out.astype(np.float32)


# revision 27
# speedup vs baseline: 1.1071x; 1.1071x over previous
"""Trainium2 Bass kernel for nn_NoPropDTEncoder (8-core data-parallel over batch).

v3 — fp8-resident redesign vs v2:
  - v2 re-streamed x from DRAM scratch twice per block (50MB/block); all 16
    DMA queues were ~85% busy -> the kernel was HBM-bound (~300MB traffic).
  - v3 keeps x^T resident in SBUF as fp8e4 ([128, 3, 2, 32, 512], 96KB/part)
    for the scores pass, plus the last 2 groups of the s-major layout
    (24KB/part); only 6 groups of s-major fp8 x (9.4MB) stream per block.
    Total DMA drops to ~50MB f32 in + 9.4MB scratch write + 4x9.4MB reads.
  - attention matmuls (scores u.x and weighted-sum att.x) run in fp8 with
    MatmulPerfMode.DoubleRow (2 k-tiles per pass, 0.5 cycles/row): both
    operands fp8e4, PSUM f32.
  - value/dense path stays bf16 (wv, wo, classifier, pool) for precision;
    wq/wk/cden are fp8 (score path / tiny-output path).
  - pool scores computed on DVE tensor_tensor_reduce against a replicated
    pool_w (bf16), z0 weighted-sum in bf16 on PE with pre-normalized
    softmax weights.

Algorithm notes (same math as v2):
  - Single-token query attention: scores(b,h,s) = u_{b,h} . x_{b,s} with
    u = wk_h^T q_h; output needs only c = att-weighted sum of x rows.
  - FFT denoise (16 modes, per-mode complex scale) folded on host with
    proj_w into one 768x768 matrix cden.
  - eval-mode BN / biases folded on host.
"""
import sys
import os
import math

for _p in ("/opt/trn_rl_repo",):
    if _p not in sys.path and os.path.isdir(_p):
        sys.path.insert(0, _p)

import numpy as np
import concourse.bass as bass
import concourse.mybir as mybir
from concourse import tile
from concourse.bass_utils import run_bass_kernel_spmd

F32 = mybir.dt.float32
BF16 = mybir.dt.bfloat16
FP8 = mybir.dt.float8e4
WDT = BF16
DR = mybir.MatmulPerfMode.DoubleRow

B, S, D, H, DH, T, NCLS = 256, 512, 768, 8, 96, 4, 14
NCORES = 8
BL = B // NCORES          # 32 batch rows per core
DBLK = D // 128           # 6
SCH = S // 128            # 4
GB = 4                    # rows per group (PSUM col-packing at 32*b)
NG = BL // GB             # 8 groups
NRES = 0                  # groups kept resident in SBUF (the last ones)
NSTR = NG - NRES          # streamed groups
GBH = 2                   # rows per pass-0 half-group (SBUF pressure)
NGH = BL // GBH           # 16 half-groups
GP = 32 * (GB - 1) + H    # 104 live partitions in packed tiles
EPS = 1e-5
RSQD = 1.0 / math.sqrt(DH)

AF = mybir.ActivationFunctionType
ALU = mybir.AluOpType

VKEYS = ("bo_sa", "projb_sd", "ln1_g", "ln1_b", "ln2_g", "ln2_b",
         "bn_s", "bn_b")


def split_sync_waits(nc, cap=1):
    """walrus in this container rejects >1 sync wait per CTRL instruction;
    move extra waits onto NoOp carriers inserted just before (same engine,
    program order => semantically identical)."""
    for f in nc.m.functions:
        for bb in f.blocks:
            il = bb.instructions
            i = 0
            while i < len(il):
                inst = il[i]
                si = inst.sync_info
                nw = len(si.on_wait) if si is not None else 0
                if nw > cap:
                    waits = list(si.on_wait)
                    ups = list(si.on_update)
                    extra, keep = waits[:-cap], waits[-cap:]
                    for j in range(0, len(extra), cap):
                        nop = mybir.InstNoOp(
                            name=f"{inst.name}-sw{j}", engine=inst.engine,
                            sync_info=mybir.SyncInfo(on_wait=extra[j:j + cap],
                                                     on_update=[]),
                            ins=[], outs=[])
                        il.insert(i, nop)
                        i += 1
                    inst.sync_info = mybir.SyncInfo(on_wait=keep, on_update=ups)
                i += 1


# ---------------------------------------------------------------------------
# host-side weight preprocessing
# ---------------------------------------------------------------------------

def _prep_weights(inp):
    f = np.float32
    w = {}
    wq = np.asarray(inp["w_q"], f)
    wk = np.asarray(inp["w_k"], f)
    wv = np.asarray(inp["w_v"], f)
    wo = np.asarray(inp["w_o"], f)
    bqkv = np.asarray(inp["b_qkv"], f)
    proj_w = np.asarray(inp["proj_w"], f)
    proj_b = np.asarray(inp["proj_b"], f)
    sa = np.asarray(inp["scale_attn"], f)
    sd = np.asarray(inp["scale_denoise"], f)

    w["wqT"] = np.ascontiguousarray(
        np.transpose(wq, (0, 2, 1)) * RSQD).astype(f)           # [T, j, i]
    bq = bqkv[:, :D] * RSQD                                     # [T, 768]
    w["bq"] = np.ascontiguousarray(
        bq.reshape(T, H, DH).transpose(0, 2, 1)).astype(f)      # [T, 96, 8]
    w["wk"] = np.ascontiguousarray(wk).astype(f)                # [T, (h i), j]
    w["wvT"] = np.ascontiguousarray(np.transpose(wv, (0, 2, 1)))
    w["woT"] = np.ascontiguousarray(np.transpose(wo, (0, 2, 1)))  # [T, m, k]
    bv = bqkv[:, 2 * D:]
    bo_eff = np.asarray(inp["b_o"], f) + np.einsum("tkm,tm->tk", wo, bv)
    bo_sa = (sa[:, None] * bo_eff).astype(f)                    # [T, 768]

    # denoise: z_spatial = z1 @ M_t ; fold with proj -> C = M @ proj_w.T
    sr = np.asarray(inp["scale_real"], np.float64)
    si = np.asarray(inp["scale_imag"], np.float64)
    filt = np.mean(sr + 1j * si, axis=2)                        # [T, 16]
    jj = np.arange(D)
    mm = np.arange(16)
    W1 = np.exp(-2j * np.pi * np.outer(jj, mm) / D)             # [768, 16]
    W2 = np.exp(+2j * np.pi * np.outer(mm, jj) / D)             # [16, 768]
    cden = np.empty((T, D, D), np.float32)
    for t in range(T):
        Mt = np.real(W1 @ (filt[t][:, None] * W2)) / D          # [j, j']
        cden[t] = (Mt @ proj_w[t].T.astype(np.float64)).astype(np.float32)
    w["cden"] = np.ascontiguousarray(cden * sd[:, None, None])  # [T, j, k]
    projb_sd = (sd[:, None] * proj_b).astype(f)

    s = 1.0 / math.sqrt(1.0 + EPS)
    vec = {
        "bo_sa": bo_sa, "projb_sd": projb_sd,
        "ln1_g": np.asarray(inp["ln1_g"], f), "ln1_b": np.asarray(inp["ln1_b"], f),
        "ln2_g": np.asarray(inp["ln2_g"], f), "ln2_b": np.asarray(inp["ln2_b"], f),
        "bn_s": (np.asarray(inp["bn_g"], f) * s).astype(f),
        "bn_b": np.asarray(inp["bn_b"], f),
    }
    w["vecs8"] = np.ascontiguousarray(
        np.stack([vec[k] for k in VKEYS], axis=1))              # [T, 8, 768]

    f1s = (np.asarray(inp["f1_bn_g"], f) * s).astype(f)
    w["f1w"] = np.asarray(inp["f1_w"], f)
    w["f1s"] = f1s
    w["f1b"] = (np.asarray(inp["f1_b"], f) * f1s + np.asarray(inp["f1_bn_b"], f))
    f2s = (np.asarray(inp["f2_bn_g"], f) * s).astype(f)
    w["f2w"] = np.asarray(inp["f2_w"], f)
    w["f2s"] = f2s
    w["f2b"] = (np.asarray(inp["f2_b"], f) * f2s + np.asarray(inp["f2_bn_b"], f))
    w["clsw"] = np.asarray(inp["cls_w"], f)
    w["clsb"] = np.asarray(inp["cls_b"], f)

    w["poolwrep"] = np.ascontiguousarray(
        np.tile(np.asarray(inp["pool_w"], f).reshape(1, D), (128, 1)))
    w["identb"] = np.eye(128, dtype=np.float32)

    g = 1.0 / (1.0 + np.exp(-np.asarray(inp["gate"], np.float64)))
    scal = {"g": [float(v) for v in g],
            "sa": [float(v) for v in sa],
            "sd": [float(v) for v in sd]}
    return w, scal


def _np_dt(dt):
    return mybir.dt.np(dt)


# ---------------------------------------------------------------------------
# program builder
# ---------------------------------------------------------------------------

def build_program(scal, for_sim=False):
    nc = bass.Bass()
    P = {}
    P["x"] = nc.declare_dram_parameter("x", [BL, S, D], F32, isOutput=False)
    P["identb"] = nc.declare_dram_parameter("identb", [128, 128], BF16, isOutput=False)
    P["poolwrep"] = nc.declare_dram_parameter("poolwrep", [128, D], BF16, isOutput=False)
    P["wqT"] = nc.declare_dram_parameter("wqT", [T, D, D], FP8, isOutput=False)
    P["bq"] = nc.declare_dram_parameter("bq", [T, DH, H], F32, isOutput=False)
    P["wk"] = nc.declare_dram_parameter("wk", [T, D, D], FP8, isOutput=False)
    P["wvT"] = nc.declare_dram_parameter("wvT", [T, D, D], WDT, isOutput=False)
    P["woT"] = nc.declare_dram_parameter("woT", [T, D, D], WDT, isOutput=False)
    P["cden"] = nc.declare_dram_parameter("cden", [T, D, D], FP8, isOutput=False)
    P["vecs8"] = nc.declare_dram_parameter("vecs8", [T, 8, D], F32, isOutput=False)
    P["f1w"] = nc.declare_dram_parameter("f1w", [D, 256], WDT, isOutput=False)
    P["f1s"] = nc.declare_dram_parameter("f1s", [256], F32, isOutput=False)
    P["f1b"] = nc.declare_dram_parameter("f1b", [256], F32, isOutput=False)
    P["f2w"] = nc.declare_dram_parameter("f2w", [256, 128], WDT, isOutput=False)
    P["f2s"] = nc.declare_dram_parameter("f2s", [128], F32, isOutput=False)
    P["f2b"] = nc.declare_dram_parameter("f2b", [128], F32, isOutput=False)
    P["clsw"] = nc.declare_dram_parameter("clsw", [128, NCLS], WDT, isOutput=False)
    P["clsb"] = nc.declare_dram_parameter("clsb", [NCLS], F32, isOutput=False)
    P["out"] = nc.declare_dram_parameter("out", [NCLS, BL], F32, isOutput=True)

    with tile.TileContext(nc) as tc:
        _body(nc, tc, P, scal, for_sim)
    if not for_sim:
        split_sync_waits(nc)
    return nc


def _body(nc, tc, P, scal, for_sim=False):
    import contextlib

    def sim_init(ap):
        # CoreSim tracks uninitialized memory; partially-written PSUM tiles
        # whose garbage partitions are read-but-never-consumed are fine on
        # HW but assert in sim. Zero them in the sim build only.
        if for_sim:
            nc.vector.memset(ap, 0.0)
    ctx = contextlib.ExitStack()
    pool_c = ctx.enter_context(tc.tile_pool(name="consts", bufs=1))
    pool_w = ctx.enter_context(tc.tile_pool(name="weights", bufs=1))
    pool_x = ctx.enter_context(tc.tile_pool(name="x", bufs=2))
    pool_s = ctx.enter_context(tc.tile_pool(name="state", bufs=1))
    pool_t = ctx.enter_context(tc.tile_pool(name="tmp", bufs=2))
    psum = ctx.enter_context(tc.tile_pool(name="ps", bufs=2,
                                          space=bass.MemorySpace.PSUM))
    pool_d = ctx.enter_context(tc.tile_pool(name="dram", bufs=1,
                                            space=bass.MemorySpace.DRAM))

    # s-major fp8 scratch for the streamed groups (layout matches the SBUF
    # tile verbatim: [128 s_lo, b, sc, d])
    xn_scr = pool_d.tile([NSTR, 128, GB, SCH, D], FP8, tag="xn_scr")

    def dma(dst, src):
        # SWDGE: this container's walrus rejects HWDGE trigger ISA structs
        nc.gpsimd.dma_start(out=dst, in_=src)

    def dma_cast(dst, src):
        # SWDGE (gpsimd) converts dtype in flight
        nc.gpsimd.dma_start(out=dst, in_=src)

    # ---- constants -------------------------------------------------------
    identb = pool_c.tile([128, 128], BF16, tag="identb")
    dma_cast(identb[:], P["identb"][:])
    identf8 = pool_c.tile([128, 128], FP8, tag="identf8")
    nc.vector.tensor_copy(identf8[:], identb[:])
    poolwrep = pool_c.tile([128, D], BF16, tag="poolwrep")
    dma(poolwrep[:], P["poolwrep"][:])
    onescol = pool_c.tile([128, 1], F32, tag="ones")
    nc.vector.memset(onescol[:], 1.0 / D)
    onescolb = pool_c.tile([128, 1], BF16, tag="onesb")
    nc.vector.memset(onescolb[:], 1.0)
    onesrow = pool_c.tile([1, 128], F32, tag="onesrow")
    nc.vector.memset(onesrow[:], 1.0)
    zeroc = pool_c.tile([128, 1], F32, tag="zeroc")
    nc.vector.memset(zeroc[:], 0.0)
    nc.const_aps.aps[(F32, 0.0)] = zeroc[:]
    epsc = pool_c.tile([128, 1], F32, tag="epsc")
    nc.vector.memset(epsc[:], EPS)
    nc.const_aps.aps[(F32, EPS)] = epsc[:]

    # classifier weights (loaded once)
    f1w = pool_c.tile([128, DBLK, 256], WDT, tag="f1w")
    dma(f1w[:], P["f1w"].rearrange("(c p) n -> p c n", p=128))
    f2w = pool_c.tile([128, 2, 128], WDT, tag="f2w")
    dma(f2w[:], P["f2w"].rearrange("(c p) n -> p c n", p=128))
    clsw = pool_c.tile([128, NCLS], WDT, tag="clsw")
    dma(clsw[:], P["clsw"][:])
    f1s = pool_c.tile([128, 2], F32, tag="f1s")
    dma(f1s[:], P["f1s"].rearrange("(c p) -> p c", p=128))
    f1b = pool_c.tile([128, 2], F32, tag="f1b")
    dma(f1b[:], P["f1b"].rearrange("(c p) -> p c", p=128))
    f2s = pool_c.tile([128, 1], F32, tag="f2s")
    dma(f2s[:], P["f2s"].rearrange("(c p) -> p c", p=128))
    f2b = pool_c.tile([128, 1], F32, tag="f2b")
    dma(f2b[:], P["f2b"].rearrange("(c p) -> p c", p=128))
    clsb = pool_c.tile([NCLS, 1], F32, tag="clsb")
    dma(clsb[:], P["clsb"].rearrange("(p c) -> p c", c=1))

    # persistent state
    # x^T resident: XT[p, pr, i, b, s] = x[b, s, (2*pr+i)*128 + p]
    XT = pool_s.tile([128, 3, 2, BL, S], FP8, tag="XT")
    # s-major resident groups (the last NRES): [p, j, b_in_g, sc, d]
    xnres = (pool_s.tile([128, NRES, GB, SCH, D], FP8, tag="xnres")
             if NRES else None)
    zT = pool_s.tile([128, BL, DBLK], F32, tag="zT")
    zTb = pool_s.tile([128, BL, DBLK], WDT, tag="zTb")
    logitsT = pool_s.tile([NCLS, BL], F32, tag="logits")

    # =====================================================================
    # pass 0: stream x f32->bf16 in half-groups (2 batch rows); build fp8 XT
    # (PE transposes) + fp8 s-major scratch; attention pool -> z0 (bf16
    # path for precision).
    # =====================================================================
    p0_xn = {}
    GPH = 32 * (GBH - 1) + 1  # live packed partitions for z0 rows

    def p0_load(ci):
        rows = slice(ci * GBH, (ci + 1) * GBH)
        xn = pool_x.tile([128, GBH, SCH, D], BF16, tag="xng", bufs=2)
        nc.gpsimd.dma_start(
            out=xn[:], in_=P["x"][rows].rearrange("b (sc p) d -> p b sc d", p=128))
        p0_xn[ci] = xn

    def p0_store(ci):
        xn = p0_xn[ci]
        gi, h = ci * GBH // GB, (ci * GBH) % GB
        if gi < NSTR:
            dma_cast(xn_scr[gi][:, h:h + GBH], xn[:])  # bf16 -> fp8 out
        else:
            nc.vector.tensor_copy(xnres[:, gi - NSTR, h:h + GBH], xn[:])

    def p0_pool(ci):
        rows = slice(ci * GBH, (ci + 1) * GBH)
        xn = p0_xn[ci]
        # scores: product on DVE, free-dim reduction via Scalar accumulate
        # (TensorTensorReduce is not encodable by this container's walrus)
        pscore = pool_t.tile([128, GBH, SCH], F32, tag="pscore", bufs=1)
        for b in range(GBH):
            for sc in range(SCH):
                scr = pool_t.tile([128, D], BF16, tag="ttr_scr", bufs=2)
                nc.vector.tensor_mul(scr[:], xn[:, b, sc, :], poolwrep[:])
                nc.scalar.activation(scr[:], scr[:], AF.Identity,
                                     accum_out=pscore[:, b, sc:sc + 1])
        pes = pool_t.tile([128, GBH, SCH], BF16, tag="pes", bufs=1)
        nc.scalar.activation(pes[:], pscore[:], AF.Exp)
        # denominators: sum over s_lo partitions (PE) then over sc (DVE)
        denp = psum.tile([1, GBH, SCH], F32, tag="lnm", bufs=1, name="denp")
        nc.tensor.matmul(denp.rearrange("p b sc -> p (b sc)"), onescolb[:],
                         pes.rearrange("p b sc -> p (b sc)"),
                         start=True, stop=True)
        dcp = pool_t.tile([1, GBH, SCH], F32, tag="dcp", bufs=1)
        nc.vector.tensor_copy(dcp[:], denp[:])
        dsum = pool_t.tile([1, GBH], F32, tag="dsum", bufs=1)
        nc.vector.tensor_add(dsum[:], dcp[:, :, 0], dcp[:, :, 1])
        nc.vector.tensor_add(dsum[:], dsum[:], dcp[:, :, 2])
        nc.vector.tensor_add(dsum[:], dsum[:], dcp[:, :, 3])
        drec = pool_t.tile([1, GBH], F32, tag="drec", bufs=1)
        nc.vector.reciprocal(drec[:], dsum[:])
        bcp = psum.tile([128, GBH], F32, tag="lnm", bufs=1, name="bcp")
        nc.tensor.matmul(bcp[:], onesrow[:], drec[:], start=True, stop=True)
        bc = pool_t.tile([128, GBH], F32, tag="pbc", bufs=1)
        nc.vector.tensor_copy(bc[:], bcp[:])
        pesn = pool_t.tile([128, GBH, SCH], BF16, tag="pesn", bufs=1)
        for b in range(GBH):
            nc.vector.tensor_scalar(pesn[:, b, :], pes[:, b, :],
                                    bc[:, b:b + 1], None, op0=ALU.mult)
        # z0 rows at partitions 32*b (m=1 weighted sums, pre-normalized)
        z1p = psum.tile([GP, 512], F32, tag="ws1", name="z1p")
        z2p = psum.tile([GP, 256], F32, tag="ws2", bufs=1, name="z2p")
        sim_init(z1p[:])
        sim_init(z2p[:])
        for b in range(GBH):
            for sc in range(SCH):
                nc.tensor.matmul(z1p[32 * b:32 * b + 1, :],
                                 pesn[:, b, sc:sc + 1], xn[:, b, sc, 0:512],
                                 start=(sc == 0), stop=(sc == SCH - 1),
                                 tile_position=(0, 32 * b))
                nc.tensor.matmul(z2p[32 * b:32 * b + 1, :],
                                 pesn[:, b, sc:sc + 1], xn[:, b, sc, 512:D],
                                 start=(sc == 0), stop=(sc == SCH - 1),
                                 tile_position=(0, 32 * b))
        zn = pool_t.tile([GPH, D], BF16, tag="zn", bufs=1)
        nc.vector.tensor_copy(zn[:, 0:512], z1p[0:GPH, :])
        nc.vector.tensor_copy(zn[:, 512:D], z2p[0:GPH, :])
        ztp = psum.tile([128, DBLK, GPH + 3], BF16, tag="sc", name="ztp")
        for dc in range(DBLK):
            nc.tensor.transpose(ztp[:, dc, 0:GPH], zn[:, dc * 128:(dc + 1) * 128],
                                identb[0:GPH, 0:GPH])
        src = ztp[:, :, 0:GPH:32].rearrange("p dc b -> p b dc")  # [128, 2, 6]
        nc.vector.tensor_copy(zT[:, rows, :], src)
        nc.vector.tensor_copy(zTb[:, rows, :], src)

    # block weight loads
    wqs, wks, bqvs, vecs = {}, {}, {}, {}

    def emit_weights(t):
        wq = pool_w.tile([128, DBLK, D], FP8, tag="wq", name="wq")
        dma(wq[:], P["wqT"][t].rearrange("(c p) n -> p c n", p=128))
        wk = pool_w.tile([DH, H, D], FP8, tag="wk", name="wk")
        dma(wk[:], P["wk"][t].rearrange("(h p) n -> p h n", p=DH))
        bqv = pool_w.tile([DH, H], F32, tag="bqv")
        dma(bqv[:], P["bq"][t])
        vec = pool_w.tile([128, 8, DBLK], F32, tag="vecs")
        dma(vec[:], P["vecs8"][t].rearrange("v (c p) -> p v c", p=128))
        wqs[t], wks[t], bqvs[t], vecs[t] = wq, wk, bqv, vec

    p0_load(0)
    emit_weights(0)
    for ci in range(NGH):
        if ci + 1 < NGH:
            p0_load(ci + 1)
        p0_store(ci)
        p0_pool(ci)

    # =====================================================================
    # transformer blocks
    # =====================================================================
    # group order: resident groups first so stream DMA has lead time
    ORDER = list(range(NSTR, NG)) + list(range(NSTR))
    xn8s = {}

    def emit_load(t, gi):
        # gi is a streamed group id (< NSTR)
        xn8 = pool_x.tile([128, GB, SCH, D], FP8, tag="xn8", bufs=2)
        dma(xn8[:], xn_scr[gi])
        xn8s[(t, gi)] = xn8

    def xn_of(t, gi, pop=True):
        if gi >= NSTR:
            return xnres[:, gi - NSTR]
        return (xn8s.pop((t, gi)) if pop else xn8s[(t, gi)])[:]

    pending_cls = [None]

    for t in range(T):
        g = scal["g"][t]
        sa = scal["sa"][t]

        wq, wk, bqv, vec = wqs[t], wks[t], bqvs[t], vecs[t]

        def vslice(k, dc, vec=vec):
            return vec[:, VKEYS.index(k), dc:dc + 1]

        # prefetch the first streamed groups
        emit_load(t, 0)
        emit_load(t, 1)

        # --- q^T = wqT^T @ zT + bq  -> [96, 8, 32] ---
        qp = psum.tile([DH, H, BL], F32, tag="mm")
        for h in range(H):
            for jc in range(DBLK):
                nc.tensor.matmul(qp[:, h, :],
                                 wq[:, jc, h * DH:(h + 1) * DH],
                                 zTb[:, :, jc],
                                 start=(jc == 0), stop=(jc == DBLK - 1))
        qT = pool_t.tile([DH, H, BL], WDT, tag="qT", bufs=1)
        for h in range(H):
            nc.vector.tensor_scalar_add(qT[:, h, :], qp[:, h, :], bqv[:, h:h + 1])

        # --- u = wk_h^T q_h -> fp8 [128, 3, 2, 32, 8] (b-major for 8B-aligned
        # DoubleRow weight slices) ---
        uT = pool_t.tile([128, 3, 2, BL, H], FP8, tag="uT", bufs=1)
        for pr in range(3):
            up = psum.tile([128, 2, H, BL], F32, tag="mm")
            for i in range(2):
                dc = 2 * pr + i
                for h in range(H):
                    nc.tensor.matmul(up[:, i, h, :],
                                     wk[:, h, dc * 128:(dc + 1) * 128],
                                     qT[:, h, :], start=True, stop=True)
            nc.vector.tensor_copy(
                uT[:, pr].rearrange("p i b h -> p i h b"), up[:])

        # previous block's classifier rides the group phase (PE slack)
        if pending_cls[0] is not None:
            pending_cls[0]()
            pending_cls[0] = None

        # prefetch tail weights + next block's head weights
        wv = pool_w.tile([128, DBLK, D], WDT, tag="wv", name="wv")
        dma(wv[:], P["wvT"][t].rearrange("(c p) n -> p c n", p=128))
        wo = pool_w.tile([DH, H, D], WDT, tag="wo", name="wo")
        dma(wo[:], P["woT"][t].rearrange("(h p) n -> p h n", p=DH))
        cdn = pool_w.tile([128, DBLK, D], FP8, tag="cd", name="cdn")
        dma(cdn[:], P["cden"][t].rearrange("(c p) n -> p c n", p=128))
        if t + 1 < T:
            emit_weights(t + 1)

        # --- streaming pass over x: software-pipelined group loop ---
        cT = pool_t.tile([128, DBLK, H, BL], WDT, tag="cT", bufs=1)
        ess, dens, esTs, chs = {}, {}, {}, {}

        def emit_xt(gi):
            # block 0 only: build resident fp8 XT from the fp8 stream tiles
            # (fp8 PE transpose writes PSUM with element step 2)
            xn8 = xn_of(0, gi, pop=False)
            for bb in range(GB):
                b = gi * GB + bb
                for sc in range(SCH):
                    tp8 = psum.tile([128, DBLK, 256], FP8, tag="mm",
                                    name="tp8")
                    for dc in range(DBLK):
                        nc.tensor.transpose(
                            tp8[:, dc, 0:256:2],
                            xn8[:, bb, sc, dc * 128:(dc + 1) * 128],
                            identf8[:])
                    dst = XT[:, :, :, b, sc * 128:(sc + 1) * 128]
                    src = tp8[:, :, 0:256:2].rearrange(
                        "p (pr i) s -> p pr i s", pr=3)
                    if (bb * SCH + sc) % 2 == 0:
                        nc.vector.tensor_copy(dst, src)
                    else:
                        nc.scalar.activation(dst, src, AF.Identity)

        def emit_scores(gi):
            ps = psum.tile([GP, S], F32, tag="sc")
            sim_init(ps[:])
            for bb in range(GB):
                b = gi * GB + bb
                for dc in range(DBLK):
                    nc.tensor.matmul(ps[32 * bb:32 * bb + H, :],
                                     uT[:, dc >> 1, dc & 1, b, :],
                                     XT[:, dc >> 1, dc & 1, b, :],
                                     start=(dc == 0), stop=(dc == DBLK - 1),
                                     tile_position=(0, 32 * bb))
            es = pool_t.tile([GP, S], BF16, tag="es")
            den = pool_t.tile([GP, 2], F32, tag="den")
            nc.scalar.activation(es[:], ps[:], AF.Exp, accum_out=den[:, 0:1])
            nc.vector.reciprocal(den[:, 1:2], den[:, 0:1])
            ess[gi], dens[gi] = es, den

        def emit_attT(gi):
            es = ess.pop(gi)
            esTp = psum.tile([128, SCH, GP], BF16, tag="lnm", bufs=1)
            for sc in range(SCH):
                nc.tensor.transpose(esTp[:, sc, 0:GP],
                                    es[:, sc * 128:(sc + 1) * 128],
                                    identb[0:GP, 0:GP])
            esT = pool_t.tile([128, 2, 2, GP], FP8, tag="esT")
            nc.vector.tensor_copy(
                esT[:], esTp.rearrange("p (pr i) g -> p pr i g", pr=2))
            esTs[gi] = esT

        def emit_wsum(t, gi):
            xn8 = xn_of(t, gi)
            xv = xn8.rearrange("p b (pr i) d -> p b pr i d", pr=2)
            esT, den = esTs.pop(gi), dens.pop(gi)
            c1 = psum.tile([GP, 512], F32, tag="ws1")
            c2 = psum.tile([GP, 256], F32, tag="ws2", bufs=1)
            sim_init(c1[:])
            sim_init(c2[:])
            for bb in range(GB):
                for sc in range(SCH):
                    nc.tensor.matmul(c1[32 * bb:32 * bb + H, :],
                                     esT[:, sc >> 1, sc & 1, 32 * bb:32 * bb + H],
                                     xv[:, bb, sc >> 1, sc & 1, 0:512],
                                     start=(sc == 0), stop=(sc == SCH - 1),
                                     tile_position=(0, 32 * bb))
                    nc.tensor.matmul(c2[32 * bb:32 * bb + H, :],
                                     esT[:, sc >> 1, sc & 1, 32 * bb:32 * bb + H],
                                     xv[:, bb, sc >> 1, sc & 1, 512:D],
                                     start=(sc == 0), stop=(sc == SCH - 1),
                                     tile_position=(0, 32 * bb))
            ch = pool_t.tile([GP, D], BF16, tag="ch")
            nc.vector.tensor_scalar_mul(ch[:, 0:512], c1[:], den[:, 1:2])
            nc.vector.tensor_scalar_mul(ch[:, 512:D], c2[:], den[:, 1:2])
            chs[gi] = ch

        def emit_ctpT(gi):
            ch = chs.pop(gi)
            ctpp = psum.tile([128, DBLK, 128], BF16, tag="mm")
            for dc in range(DBLK):
                nc.tensor.transpose(ctpp[:, dc, 0:GP],
                                    ch[:, dc * 128:(dc + 1) * 128],
                                    identb[0:GP, 0:GP])
            # scatter (dc, 32*bb+h) -> cT[:, dc, h, gi*GB+bb]
            src = ctpp.rearrange("p dc (b r) -> p dc b r", r=32)[:, :, :, 0:H]
            nc.vector.tensor_copy(
                cT.rearrange("p dc h bl -> p dc bl h")[:, :, gi * GB:(gi + 1) * GB, :],
                src)

        for j, gi in enumerate(ORDER):
            if t == 0:
                emit_xt(gi)
            emit_scores(gi)
            if j >= 1:
                emit_wsum(t, ORDER[j - 1])
            emit_attT(gi)
            if j >= 1:
                emit_ctpT(ORDER[j - 1])
            if j + 2 < NG and j + 2 >= NRES:
                emit_load(t, ORDER[j + 2])
        emit_wsum(t, ORDER[NG - 1])
        emit_ctpT(ORDER[NG - 1])

        # --- o_h = wvT_h^T @ c_h -> [96, 8, 32] bf16 ---
        op = psum.tile([DH, H, BL], F32, tag="mm")
        for h in range(H):
            for jc in range(DBLK):
                nc.tensor.matmul(op[:, h, :],
                                 wv[:, jc, h * DH:(h + 1) * DH],
                                 cT[:, jc, h, :],
                                 start=(jc == 0), stop=(jc == DBLK - 1))
        oT = pool_t.tile([DH, H, BL], WDT, tag="oT", bufs=1)
        nc.vector.tensor_copy(oT[:], op[:])

        # --- z_attn^T = woT^T @ o ; y = z + sa*z_attn + sa*bo_eff ---
        zap = psum.tile([128, DBLK, BL], F32, tag="mm")
        for mk in range(DBLK):
            for h in range(H):
                nc.tensor.matmul(zap[:, mk, :],
                                 wo[:, h, mk * 128:(mk + 1) * 128],
                                 oT[:, h, :], start=(h == 0), stop=(h == H - 1))
        yT = pool_s.tile([128, BL, DBLK], F32, tag="yT")
        for mk in range(DBLK):
            nc.vector.tensor_scalar(yT[:, :, mk], zap[:, mk, :],
                                    sa, vslice("bo_sa", mk),
                                    op0=ALU.mult, op1=ALU.add)
        nc.vector.tensor_add(yT[:], yT[:], zT[:])

        # --- LN1 -> z1T ---
        z1T = pool_s.tile([128, BL, DBLK], F32, tag="z1T")
        _layernorm(nc, tc, psum, pool_t, yT, z1T, onescol, onesrow,
                   lambda dc: vslice("ln1_g", dc), lambda dc: vslice("ln1_b", dc))

        # --- denoise: z_den^T = cden^T @ z1T ; y2 = z1 + sd*(...) ---
        z1Tb = pool_t.tile([128, BL, DBLK], WDT, tag="z1Tb", bufs=1)
        nc.vector.tensor_copy(z1Tb[:], z1T[:])
        dp = psum.tile([128, DBLK, BL], F32, tag="mm")
        for mk in range(DBLK):
            for jc in range(DBLK):
                nc.tensor.matmul(dp[:, mk, :],
                                 cdn[:, jc, mk * 128:(mk + 1) * 128],
                                 z1Tb[:, :, jc],
                                 start=(jc == 0), stop=(jc == DBLK - 1))
        y2T = pool_s.tile([128, BL, DBLK], F32, tag="yT")  # reuse yT slot
        for mk in range(DBLK):
            nc.vector.tensor_scalar_add(y2T[:, :, mk], dp[:, mk, :],
                                        vslice("projb_sd", mk))
        nc.vector.tensor_add(y2T[:], y2T[:], z1T[:])

        # --- LN2 -> z2T ---
        z2T = pool_s.tile([128, BL, DBLK], F32, tag="z1T")  # reuse z1T slot
        _layernorm(nc, tc, psum, pool_t, y2T, z2T, onescol, onesrow,
                   lambda dc: vslice("ln2_g", dc), lambda dc: vslice("ln2_b", dc))

        # --- gate mix + BN -> new z ---
        nc.vector.tensor_sub(z2T[:], z2T[:], zT[:])       # z2 - z
        nc.vector.tensor_scalar(z2T[:], z2T[:], g, None,
                                op0=ALU.mult)              # g*(z2-z)
        nc.vector.tensor_add(z2T[:], z2T[:], zT[:])       # + z
        for dc in range(DBLK):
            nc.vector.tensor_scalar(zT[:, :, dc], z2T[:, :, dc],
                                    vslice("bn_s", dc), vslice("bn_b", dc),
                                    op0=ALU.mult, op1=ALU.add)
        nc.vector.tensor_copy(zTb[:], zT[:])

        # --- classifier (deferred: emitted during block t+1's group phase) ---
        def emit_classifier(t=t):
            hp = psum.tile([128, 2, BL], F32, tag="mm")
            for mk in range(2):
                for jc in range(DBLK):
                    nc.tensor.matmul(hp[:, mk, :],
                                     f1w[:, jc, mk * 128:(mk + 1) * 128],
                                     zTb[:, :, jc],
                                     start=(jc == 0), stop=(jc == DBLK - 1))
            h1 = pool_t.tile([128, 2, BL], WDT, tag="h1", bufs=1)
            for mk in range(2):
                nc.scalar.activation(h1[:, mk, :], hp[:, mk, :], AF.Relu,
                                     bias=f1b[:, mk:mk + 1],
                                     scale=f1s[:, mk:mk + 1])
            h2p = psum.tile([128, BL], F32, tag="mm")
            for jc in range(2):
                nc.tensor.matmul(h2p[:], f2w[:, jc, :], h1[:, jc, :],
                                 start=(jc == 0), stop=(jc == 1))
            h2 = pool_t.tile([128, BL], WDT, tag="h2", bufs=1)
            nc.scalar.activation(h2[:], h2p[:], AF.Relu,
                                 bias=f2b[:, 0:1], scale=f2s[:, 0:1])
            lp = psum.tile([NCLS, BL], F32, tag="mm")
            nc.tensor.matmul(lp[:], clsw[:], h2[:], start=True, stop=True)
            if t == 0:
                nc.vector.tensor_copy(logitsT[:], lp[:])
            else:
                nc.vector.tensor_add(logitsT[:], logitsT[:], lp[:])

        pending_cls[0] = emit_classifier

    pending_cls[0]()
    pending_cls[0] = None

    # --- epilogue: /T + cls_b, store ---
    outt = pool_t.tile([NCLS, BL], F32, tag="outt")
    nc.scalar.activation(outt[:], logitsT[:], AF.Identity,
                         bias=clsb[:, 0:1], scale=1.0 / T)
    dma_cast(P["out"][:], outt[:])
    ctx.close()


def _layernorm(nc, tc, psum, pool_t, yT, outT, onescol, onesrow, gf, bf):
    """T-layout layernorm over d (partition x dblk); DVE + PE reductions."""
    mp = psum.tile([1, BL], F32, tag="lnm", bufs=1, name="mp")
    m2p = psum.tile([1, BL], F32, tag="mm", name="m2p")
    sq = pool_t.tile([128, BL, DBLK], F32, tag="ln_sq", bufs=1)
    nc.vector.tensor_mul(sq[:], yT[:], yT[:])
    for dc in range(DBLK):
        nc.tensor.matmul(mp[:], onescol[:], yT[:, :, dc],
                         start=(dc == 0), stop=(dc == DBLK - 1))
        nc.tensor.matmul(m2p[:], onescol[:], sq[:, :, dc],
                         start=(dc == 0), stop=(dc == DBLK - 1))
    st = pool_t.tile([1, 2 * BL], F32, tag="ln_st", bufs=1)  # [mu | rstd]
    nc.vector.tensor_copy(st[:, 0:BL], mp[:])
    mu2 = pool_t.tile([1, BL], F32, tag="ln_mu2", bufs=1)
    nc.vector.tensor_mul(mu2[:], st[:, 0:BL], st[:, 0:BL])
    var = pool_t.tile([1, BL], F32, tag="ln_var", bufs=1)
    nc.vector.tensor_sub(var[:], m2p[:], mu2[:])
    nc.scalar.activation(var[:], var[:], AF.Sqrt, bias=EPS)
    nc.vector.reciprocal(st[:, BL:2 * BL], var[:])
    bcp = psum.tile([128, 2 * BL], F32, tag="lnm", bufs=1)
    nc.tensor.matmul(bcp[:], onesrow[:], st[:], start=True, stop=True)
    bc = pool_t.tile([128, 2 * BL], F32, tag="ln_bc", bufs=1)
    nc.vector.tensor_copy(bc[:], bcp[:])
    mub = bc[:, 0:BL]
    rsb = bc[:, BL:2 * BL]
    for dc in range(DBLK):
        nc.vector.tensor_sub(outT[:, :, dc], yT[:, :, dc], mub)
        nc.vector.tensor_mul(outT[:, :, dc], outT[:, :, dc], rsb)
        nc.vector.tensor_scalar(outT[:, :, dc], outT[:, :, dc],
                                gf(dc), bf(dc), op0=ALU.mult, op1=ALU.add)


# ---------------------------------------------------------------------------
# entry point
# ---------------------------------------------------------------------------

_PROG_CACHE = {}


def _make_in_maps(inputs, w, scal):
    x = np.asarray(inputs["x_feat"], np.float32)
    assert x.shape == (B, S, D), x.shape

    def cast(a, dt):
        return np.ascontiguousarray(a).astype(_np_dt(dt))

    shared = {
        "identb": cast(w["identb"], BF16),
        "poolwrep": cast(w["poolwrep"], BF16),
        "wqT": cast(w["wqT"], FP8), "bq": w["bq"], "wk": cast(w["wk"], FP8),
        "wvT": cast(w["wvT"], WDT), "woT": cast(w["woT"], WDT),
        "cden": cast(w["cden"], FP8),
        "vecs8": w["vecs8"],
        "f1w": cast(w["f1w"], WDT), "f1s": w["f1s"], "f1b": w["f1b"],
        "f2w": cast(w["f2w"], WDT), "f2s": w["f2s"], "f2b": w["f2b"],
        "clsw": cast(w["clsw"], WDT), "clsb": w["clsb"],
    }
    in_maps = []
    for c in range(NCORES):
        m = dict(shared)
        m["x"] = np.ascontiguousarray(x[c * BL:(c + 1) * BL])
        in_maps.append(m)
    return in_maps


LAST_EXEC_NS = None
LAST_RESULTS = None


def kernel(**inputs):
    global LAST_EXEC_NS, LAST_RESULTS
    w, scal = _prep_weights(inputs)
    key = tuple(scal["g"]) + tuple(scal["sa"]) + tuple(scal["sd"])
    if key not in _PROG_CACHE:
        _PROG_CACHE[key] = build_program(scal)
    nc = _PROG_CACHE[key]
    in_maps = _make_in_maps(inputs, w, scal)
    res = run_bass_kernel_spmd(nc, in_maps, core_ids=list(range(NCORES)))
    LAST_RESULTS = res
    if res.exec_time_ns:
        LAST_EXEC_NS = res.exec_time_ns
    out = np.concatenate(
        [np.asarray(res.results[c]["out"]).T for c in range(NCORES)], axis=0)
    return out.astype(np.float32)
